# revision 1
# baseline (speedup 1.0000x reference)
"""Trainium2 Bass kernel for nn_DenseAttentionLayer (gnn_message_passing) — v3.

Math (reference):
    in_fts = context @ W_common.T            # (N, HID)
    left   = in_fts @ w_left + b_left        # (N,)
    right  = in_fts @ w_right + b_right      # (N,)
    logits = leaky_relu(left[:,None] + right[None,:], 0.2)
    logits = where(adj <= 0, -inf, logits)
    coefs  = softmax(logits, axis=-1)
    out    = relu(coefs @ relation)          # (N, REL_DIM)

Design:
  * TRANSPOSED elementwise layout: j on partitions, i on the free dim.
    The host uploads adj^T per core, so zm is produced directly in the
    lhsT orientation the P@V matmul needs -> no PE transposes, no
    PSUM->SBUF copies, no 4MB right-broadcast.
  * Two per-tile formulations, mixed at a g-fraction to balance ACT/DVE:
    - ACT form: zm = adj * Exp(Prelu(left_bcast + r_j_bias, alpha=0.2))
      — Prelu honors alpha on this HW and shares an act-table set with
      Exp, so exp(leaky(l+r)) is two fused-bias ACT ops; mask on DVE.
    - DVE form (rank-1 factorization): exp(x), x = l_i + r_j factorizes
      as e^l * e^r; with A=e^l, B=e^r, C=e^{0.2l}, D=e^{0.2r},
        zm_ij = adj_ij * max(A_i*B_j, C_i*D_j)
      = two 4x-mode tensor_scalar products + tensor_max + mask (2x).
  * DOT-PRODUCTS ON PE: context is uploaded transposed (ctxT [IN, N]),
    so right_j = ctx_j . v_right becomes 4 accumulating [128,128]x[128,1]
    matmuls per j-tile with the contraction on partitions — freeing the
    DVE/ACT engines (TensorTensor/TensorScalar are DVE-only on this
    toolchain; GPSIMD cannot run them).
  * PSUM packing: per-bank tiles. 4 banks hold 8 packed 256-wide output
    accumulators (two i-blocks per [128,512] bank tile), 1 bank holds
    the 8 softmax denominators (8 width-1 matmuls per j-tile against a
    ones column), 3 banks rotate for the PE dot-product chunks.

Sharding: row-shard the N x N logits; core c owns output rows
[c*R, (c+1)*R), R = N/8.  All params + full context replicated.
"""

import os
import sys

for _p in ("/opt/trn_rl_repo",):
    if _p not in sys.path and os.path.isdir(_p):
        sys.path.insert(0, _p)

from contextlib import ExitStack

import ml_dtypes
import numpy as np

# ---------------------------------------------------------------- constants
N = 8192  # num relations (columns j)
IN = 512  # context feature dim
D = 256  # relation dim (output dim)
NCORES = 8
P = 128
R = N // NCORES  # rows per core (i range)
KT = IN // P  # IN tiles (4)

_CACHE = {}


def _spread(frac, n):
    """n booleans with ~frac True, evenly spread."""
    out = []
    acc = 0.0
    for _ in range(n):
        acc += frac
        if acc >= 1.0 - 1e-9:
            acc -= 1.0
            out.append(True)
        else:
            out.append(False)
    return out


# ------------------------------------------------------------------ builder
def build_program(cfg):
    import concourse.bass as bass
    import concourse.tile as tile
    from concourse import bacc, mybir

    f32 = mybir.dt.float32
    bf16 = mybir.dt.bfloat16

    reps = cfg.get("reps", 1)
    g_act = cfg.get("g_act", 0.54)  # fraction of tiles on ACT (Prelu+Exp)
    lookahead = cfg.get("lookahead", 16)  # dot j-tiles emitted ahead
    njt = N // P  # 64 j-tiles
    NCH = 8  # j-tiles per dot chunk
    nch = njt // NCH  # 8 chunks
    ni = R // P  # 8 i-blocks
    NREL = 8  # rel load chunks

    Exp = mybir.ActivationFunctionType.Exp
    Relu = mybir.ActivationFunctionType.Relu
    Copy = mybir.ActivationFunctionType.Copy
    Prelu = mybir.ActivationFunctionType.Prelu
    Alu = mybir.AluOpType

    nc = bacc.Bacc("TRN2", target_bir_lowering=False, debug=False)

    adjt = nc.dram_tensor("adjt", [N, R], bf16, kind="ExternalInput")
    ctxt = nc.dram_tensor("ctxt", [IN, N], bf16, kind="ExternalInput")
    ctxot = nc.dram_tensor("ctxot", [IN, R], bf16, kind="ExternalInput")
    rel_in = nc.dram_tensor("rel_in", [N, D], bf16, kind="ExternalInput")
    vl_in = nc.dram_tensor("vl_in", [IN], bf16, kind="ExternalInput")
    vr_in = nc.dram_tensor("vr_in", [IN], bf16, kind="ExternalInput")
    # bias2 = [b_l + b_r, 0.2*(b_l + b_r)]
    bias2 = nc.dram_tensor("bias2", [2], f32, kind="ExternalInput")
    out = nc.dram_tensor("out", [R, D], f32, kind="ExternalOutput")
    l_scr = nc.dram_tensor("l_scr", [R], f32)
    a_scr = nc.dram_tensor("a_scr", [R], bf16)
    c_scr = nc.dram_tensor("c_scr", [R], bf16)

    t_sched = _spread(g_act, njt)  # True -> ACT-form tile (Prelu+Exp)

    with tile.TileContext(nc) as tc, ExitStack() as ctx:
        singles = ctx.enter_context(tc.tile_pool(name="singles", bufs=1))
        ctx_pool = ctx.enter_context(tc.tile_pool(name="ctxp", bufs=cfg.get("ctx_bufs", 5)))
        adj_pool = ctx.enter_context(tc.tile_pool(name="adjp", bufs=cfg.get("adj_bufs", 5)))
        rc_pool = ctx.enter_context(tc.tile_pool(name="rcp", bufs=cfg.get("rc_bufs", 7)))
        t_pool = ctx.enter_context(tc.tile_pool(name="tp", bufs=cfg.get("t_bufs", 6)))
        lk_pool = ctx.enter_context(tc.tile_pool(name="lkp", bufs=cfg.get("lk_bufs", 4)))
        m_pool_t = ctx.enter_context(tc.tile_pool(name="mp", bufs=cfg.get("m_bufs", 6)))
        zm_pool = ctx.enter_context(tc.tile_pool(name="zmp", bufs=cfg.get("zm_bufs", 8)))
        out_pool = ctx.enter_context(tc.tile_pool(name="outp", bufs=4))
        sm_pool = ctx.enter_context(tc.tile_pool(name="smp", bufs=4))
        acc_psum = ctx.enter_context(
            tc.tile_pool(name="accps", bufs=1, space="PSUM")
        )
        dot_psum = ctx.enter_context(
            tc.tile_pool(name="dotps", bufs=3, space="PSUM")
        )

        def _emit_body():
            # ---------------- singles / params ----------------
            # v_left / v_right in column layout [128, KT] (per-IN-tile cols)
            vlT = singles.tile([P, KT], bf16)
            nc.sync.dma_start(
                out=vlT, in_=bass.AP(tensor=vl_in, offset=0, ap=[[1, P], [P, KT]])
            )
            vrT = singles.tile([P, KT], bf16)
            nc.sync.dma_start(
                out=vrT, in_=bass.AP(tensor=vr_in, offset=0, ap=[[1, P], [P, KT]])
            )
            b2 = singles.tile([P, 2], f32)
            nc.sync.dma_start(
                out=b2, in_=bass.AP(tensor=bias2, offset=0, ap=[[0, P], [1, 2]])
            )
            ones_col = singles.tile([P, 1], bf16)
            nc.vector.memset(ones_col[:], 1.0)
            # warm the ACT function table at t~0 (the implicit
            # LoadActFuncSet otherwise lands mid-startup-chain, adding
            # ~1.3us to the first Prelu's latency)
            warm = singles.tile([P, 1], bf16)
            nc.scalar.activation(warm, ones_col, Exp, bias=0.0, scale=1.0)
            zeros_sb = singles.tile([P, 2 * D], bf16)
            nc.vector.memset(zeros_sb[:], 0.0)
            zid = singles.tile([P, P], bf16)
            nc.vector.memset(zid[:], 0.0)

            # relation tiles [P, njt, D], loaded in NREL chunks
            rel_sb = singles.tile([P, njt, D], bf16)
            relw = njt // NREL

            def emit_rel_chunk(rq):
                nc.sync.dma_start(
                    out=rel_sb[:, rq * relw : (rq + 1) * relw, :],
                    in_=bass.AP(tensor=rel_in, offset=rq * relw * P * D,
                                ap=[[D, P], [P * D, relw], [1, D]]),
                )

            # ---------------- right-dot machinery (PE) ----------------
            # ctx chunk q covers j-tiles [q*NCH, (q+1)*NCH)
            ctx_tiles = {}
            r_chunks = {}

            def emit_ctx_chunk(q):
                ct = ctx_pool.tile([P, KT, NCH * P], bf16, tag="ctx", name="ct")
                nc.sync.dma_start(
                    out=ct,
                    in_=bass.AP(
                        tensor=ctxt,
                        offset=q * NCH * P,
                        ap=[[N, P], [P * N, KT], [1, NCH * P]],
                    ),
                )
                ctx_tiles[q] = ct
                r_ps = dot_psum.tile([P, NCH], f32, tag="rdot", name="r_ps")
                r_chunks[q] = [r_ps, None, None, None]

            def emit_dot_tile(j):
                q, t = j // NCH, j % NCH
                if t == 0 and q not in ctx_tiles:
                    emit_ctx_chunk(q)
                r_ps = r_chunks[q][0]
                ct = ctx_tiles[q]
                for kt in range(KT):
                    nc.tensor.matmul(
                        r_ps[:, t : t + 1],
                        lhsT=ct[:, kt, t * P : (t + 1) * P],
                        rhs=vrT[:, kt : kt + 1],
                        start=(kt == 0),
                        stop=(kt == KT - 1),
                    )
                if t == NCH - 1:
                    # chunk complete: pull to SBUF + derive exp factors (ACT)
                    r_col = rc_pool.tile([P, NCH], f32, tag="rcol", name="r_col")
                    nc.scalar.activation(r_col, r_ps, Copy, bias=0.0, scale=1.0)
                    B_col = rc_pool.tile([P, NCH], f32, tag="bcol", name="B_col")
                    nc.scalar.activation(B_col, r_ps, Exp, bias=0.0, scale=1.0)
                    D_col = rc_pool.tile([P, NCH], f32, tag="dcol", name="D_col")
                    nc.scalar.activation(D_col, r_ps, Exp, bias=0.0, scale=0.2)
                    r_chunks[q][1:] = [r_col, B_col, D_col]

            def emit_adjt(jt):
                at = adj_pool.tile([P, 4, R], bf16, tag="adj", name="at")
                nc.sync.dma_start(
                    out=at,
                    in_=bass.AP(
                        tensor=adjt,
                        offset=jt * P * R,
                        ap=[[R, P], [P * R, 4], [1, R]],
                    ),
                )
                return at

            # ---- prefetch input streams (pure DMAs, nothing waits) ----
            own_ctx = singles.tile([P, KT, R], bf16)
            nc.sync.dma_start(
                out=own_ctx,
                in_=bass.AP(tensor=ctxot, offset=0,
                            ap=[[R, P], [P * R, KT], [1, R]]),
            )
            emit_ctx_chunk(0)
            adjt_tiles = {0: emit_adjt(0)}
            emit_rel_chunk(0)
            emit_ctx_chunk(1)
            adjt_tiles[1] = emit_adjt(4)
            emit_rel_chunk(1)

            # ---------------- left factors (own rows, PE dots) ----------
            l_ps = dot_psum.tile([P, ni], f32, tag="rdot", name="l_ps")
            for t in range(ni):
                for kt in range(KT):
                    nc.tensor.matmul(
                        l_ps[:, t : t + 1],
                        lhsT=own_ctx[:, kt, t * P : (t + 1) * P],
                        rhs=vlT[:, kt : kt + 1],
                        start=(kt == 0),
                        stop=(kt == KT - 1),
                    )
            # left = dot + b_left + b_right (both biases folded on the i side)
            left_col = singles.tile([P, ni], f32)
            nc.vector.tensor_scalar_add(left_col, l_ps, b2[:, 0:1])
            # tiny per-column exps, then bounce all three i-indexed vectors
            # to DRAM and broadcast back along partitions (keeps the big ACT
            # engine out of the startup critical path)
            A_col = singles.tile([P, ni], bf16)
            nc.scalar.activation(A_col, left_col, Exp, bias=0.0, scale=1.0)
            C_col = singles.tile([P, ni], bf16)
            nc.scalar.activation(C_col, left_col, Exp, bias=0.0, scale=0.2)
            nc.sync.dma_start(
                out=bass.AP(tensor=l_scr, offset=0, ap=[[1, P], [P, ni]]),
                in_=left_col[:, 0:ni],
            )
            nc.sync.dma_start(
                out=bass.AP(tensor=a_scr, offset=0, ap=[[1, P], [P, ni]]),
                in_=A_col[:, 0:ni],
            )
            nc.sync.dma_start(
                out=bass.AP(tensor=c_scr, offset=0, ap=[[1, P], [P, ni]]),
                in_=C_col[:, 0:ni],
            )
            left_bcast = singles.tile([P, R], f32)
            nc.sync.dma_start(
                out=left_bcast,
                in_=bass.AP(tensor=l_scr, offset=0, ap=[[0, P], [1, R]]),
            )
            A_bcast = singles.tile([P, R], bf16)
            nc.sync.dma_start(
                out=A_bcast,
                in_=bass.AP(tensor=a_scr, offset=0, ap=[[0, P], [1, R]]),
            )
            C_bcast = singles.tile([P, R], bf16)
            nc.sync.dma_start(
                out=C_bcast,
                in_=bass.AP(tensor=c_scr, offset=0, ap=[[0, P], [1, R]]),
            )

            for j in range(min(lookahead, njt)):
                emit_dot_tile(j)

            # ---------------- psum accumulators (packed) ----------------
            # two 256-wide i-block accumulators per [128,512] bank tile
            accab = []
            for ph in range(ni // 2):
                t_ = acc_psum.tile([P, 2 * D], f32, tag=f"accab{ph}", name=f"accab{ph}")
                accab.append(t_)
            denoms = acc_psum.tile([P, ni], f32, tag="denoms", name="denoms")
            # pre-zero the packed banks with single whole-bank zero-matmuls;
            # the per-j-tile matmuls then accumulate (start=False) only.
            # (a start=True write into a bank corrupts other in-flight
            # accumulation chains packed in the same bank)
            for ph in range(ni // 2):
                nc.tensor.matmul(
                    accab[ph][:], lhsT=zid[:], rhs=zeros_sb[:],
                    start=True, stop=True,
                )
            nc.tensor.matmul(
                denoms[:], lhsT=zid[:], rhs=zeros_sb[:, 0:ni],
                start=True, stop=True,
            )

            def acc_region(ib):
                return accab[ib // 2][:, (ib % 2) * D : (ib % 2 + 1) * D]

            # ---------------- main loop over j-tiles ----------------
            for jt in range(njt):
                q, tq = jt // NCH, jt % NCH
                if jt + lookahead < njt:
                    emit_dot_tile(jt + lookahead)
                if jt % 4 == 0 and jt + 8 < njt:
                    adjt_tiles[jt // 4 + 2] = emit_adjt(jt + 8)
                if jt % 4 == 2 and 2 + jt // 4 < NREL:
                    emit_rel_chunk(2 + jt // 4)
                adjt_tile = adjt_tiles[jt // 4]
                _, r_col, B_col, D_col = r_chunks[q]

                if t_sched[jt]:
                    # ACT form: exp(leaky(l_i + r_j)) in two fused ACT ops —
                    # Prelu honors alpha on this hardware (unlike Lrelu)
                    lk = lk_pool.tile([P, R], f32, tag="lk", name="lk")
                    nc.scalar.activation(
                        lk, left_bcast, Prelu,
                        bias=r_col[:, tq : tq + 1], scale=1.0, alpha=0.2,
                    )
                    mt = m_pool_t.tile([P, R], bf16, tag="m", name="mt")
                    nc.scalar.activation(mt, lk, Exp, bias=0.0, scale=1.0)
                else:
                    # DVE form: max(A_i*B_j, C_i*D_j) via 4x tensor_scalar
                    ts = []
                    for k in range(2):
                        tk = t_pool.tile([P, R], bf16, tag=f"t{k}", name="tk")
                        src_b = A_bcast if k == 0 else C_bcast
                        sc = B_col if k == 0 else D_col
                        nc.vector.tensor_scalar(
                            tk, src_b, sc[:, tq : tq + 1], None, Alu.mult
                        )
                        ts.append(tk)
                    mt = m_pool_t.tile([P, R], bf16, tag="m", name="mt")
                    nc.vector.tensor_max(mt, ts[0], ts[1])

                zm = zm_pool.tile([P, R], bf16, tag="zm", name="zm")
                nc.vector.tensor_tensor(
                    zm, mt, adjt_tile[:, jt % 4, :], op=Alu.mult
                )

                for ib in range(ni):
                    lhsT = zm[:, ib * P : (ib + 1) * P]
                    nc.tensor.matmul(
                        acc_region(ib),
                        lhsT=lhsT,
                        rhs=rel_sb[:, jt, :],
                        start=False,
                        stop=(jt == njt - 1),
                    )
                    nc.tensor.matmul(
                        denoms[:, ib : ib + 1],
                        lhsT=lhsT,
                        rhs=ones_col[:],
                        start=False,
                        stop=(jt == njt - 1),
                    )

            # ---------------- finalize ----------------
            for ib in range(ni):
                recip = sm_pool.tile([P, 1], f32, tag="recip", name="recip")
                nc.vector.reciprocal(recip, denoms[:, ib : ib + 1])
                ob = out_pool.tile([P, D], f32, tag="ob", name="ob")
                nc.scalar.activation(
                    ob, acc_region(ib), Relu, bias=0.0, scale=recip[:, 0:1]
                )
                nc.sync.dma_start(out=out[ib * P : (ib + 1) * P, :], in_=ob)

        if reps > 1:
            with tc.For_i(0, reps, 1):
                _emit_body()
        else:
            _emit_body()

    nc.compile()
    return nc


_BASE_CFG = dict(g_act=0.54)


def _get_program(cfg_key):
    if cfg_key not in _CACHE:
        _CACHE[cfg_key] = build_program(dict(_BASE_CFG))
    return _CACHE[cfg_key]


def prepare_in_maps(relation, context, adj_tensor, W_common, w_left, b_left,
                    w_right, b_right):
    bf = ml_dtypes.bfloat16
    relation = np.asarray(relation, dtype=np.float32)
    context = np.asarray(context, dtype=np.float32)
    adj_tensor = np.asarray(adj_tensor, dtype=np.float32)
    W_common = np.asarray(W_common, dtype=np.float32)
    w_left = np.asarray(w_left, dtype=np.float32)
    w_right = np.asarray(w_right, dtype=np.float32)
    b_l = float(np.asarray(b_left))
    b_r = float(np.asarray(b_right))

    # host-side parameter folding (weights only, no activations)
    v_left = (W_common.T @ w_left).astype(bf)
    v_right = (W_common.T @ w_right).astype(bf)
    b2 = b_l + b_r
    bias2 = np.array([b2, 0.2 * b2], dtype=np.float32)

    relb = relation.astype(bf)
    ctx_t = np.ascontiguousarray(context.T).astype(bf)  # [IN, N]

    in_maps = []
    for c in range(NCORES):
        sl = slice(c * R, (c + 1) * R)
        in_maps.append({
            "adjt": np.ascontiguousarray(adj_tensor[sl].T).astype(bf),
            "ctxt": ctx_t,
            "ctxot": np.ascontiguousarray(ctx_t[:, sl]),
            "rel_in": relb,
            "vl_in": v_left,
            "vr_in": v_right,
            "bias2": bias2,
        })
    return in_maps


# ------------------------------------------------------------------- entry
def kernel(relation, context, adj_tensor, W_common, w_left, b_left, w_right,
           b_right):
    from concourse.bass_utils import run_bass_kernel_spmd

    in_maps = prepare_in_maps(relation, context, adj_tensor, W_common,
                              w_left, b_left, w_right, b_right)
    nc = _get_program("main")
    last_err = None
    for _attempt in range(3):
        try:
            res = run_bass_kernel_spmd(nc, in_maps, list(range(NCORES)))
            outs = [res.results[c]["out"] for c in range(NCORES)]
            return np.concatenate(outs, axis=0).astype(np.float32)
        except Exception as e:  # transient device-unrecoverable seen on axon
            last_err = e
            import time as _time

            try:
                import jax

                jax.clear_caches()
            except Exception:
                pass
            _time.sleep(3.0)
    raise last_err



# revision 11
# speedup vs baseline: 1.3169x; 1.3169x over previous
"""Trainium2 Bass kernel for nn_DenseAttentionLayer (gnn_message_passing) — v4.

Math (reference):
    in_fts = context @ W_common.T            # (N, HID)
    left   = in_fts @ w_left + b_left        # (N,)
    right  = in_fts @ w_right + b_right      # (N,)
    logits = leaky_relu(left[:,None] + right[None,:], 0.2)
    logits = where(adj <= 0, -inf, logits)
    coefs  = softmax(logits, axis=-1)
    out    = relu(coefs @ relation)          # (N, REL_DIM)

v4 design (changes vs v3):
  * Mask in LOG space for ACT-form tiles: the host encodes the adjacency
    as fp8 {0, -240}; a software-DGE CCE-ADD DMA adds it into the Prelu
    output before Exp, so exp() underflows to 0 on masked entries. The
    mask costs zero DVE/ACT work on those tiles and the adjacency
    crosses HBM as 1 byte/elem.
  * DVE-form tiles read the adjacency as {0,1} fp8 via a cast DMA
    (fp8 in HBM -> bf16 in SBUF) and apply one tensor_tensor mult.
  * Denominators ride the main matmul: the relation is augmented with a
    ones column (rhs [128, 257]); acc column 256 accumulates the
    softmax denominator. Removes 512 single-column matmuls + their
    weight loads per core.
  * left/right dot products via wide rhs matmuls (lhsT = v [128,1],
    rhs = ctx chunk [128, 512]) — 16x fewer PE instructions than
    column-form dots.
  * Right factors are computed only for the core's own 1024 columns and
    AllGathered (DRAM collective) — the replicated 8.4MB ctxt read is
    gone entirely.
  * PSUM: 8 banks hold the 8 i-block accumulators [128, 257]; the
    startup dots borrow two banks (tag-versioned) before they are
    zeroed for accumulation.

Sharding: row-shard the N x N logits; core c owns output rows
[c*R, (c+1)*R), R = N/8.  All params replicated; context sharded for
the right factors (own slice only).
"""

import os
import sys

for _p in ("/opt/trn_rl_repo",):
    if _p not in sys.path and os.path.isdir(_p):
        sys.path.insert(0, _p)

from contextlib import ExitStack

import ml_dtypes
import numpy as np

# ---------------------------------------------------------------- constants
N = 8192  # num relations (columns j)
IN = 512  # context feature dim
D = 256  # relation dim (output dim)
DA = D + 1  # + ones column (softmax denominator)
NCORES = 8
P = 128
R = N // NCORES  # rows per core (i range)
KT = IN // P  # IN tiles (4)
NJT = N // P  # 64 j-tiles
NI = R // P  # 8 i-blocks

_CACHE = {}


def _spread(frac, n):
    """n booleans with ~frac True, evenly spread."""
    out = []
    acc = 0.0
    for _ in range(n):
        acc += frac
        if acc >= 1.0 - 1e-9:
            acc -= 1.0
            out.append(True)
        else:
            out.append(False)
    return out


def t_sched_for(cfg):
    if cfg.get("group4", False):
        # group-aligned schedule: groups of 4 j-tiles share one form so a
        # single SWDGE trigger covers the whole group
        g = _spread(cfg.get("g_act", 0.45), NJT // 4)
        return [v for v in g for _ in range(4)]
    return _spread(cfg.get("g_act", 0.45), NJT)


# ------------------------------------------------------------------ builder
def build_program(cfg):
    import concourse.bass as bass
    import concourse.tile as tile
    from concourse import bacc, mybir

    f32 = mybir.dt.float32
    bf16 = mybir.dt.bfloat16
    fp8 = mybir.dt.float8e4

    reps = cfg.get("reps", 1)
    adj_ahead = cfg.get("adj_ahead", 6)  # j-tiles of adj prefetch for F3
    nrel = cfg.get("nrel", 8)  # rel load chunks
    relw = NJT // nrel

    Exp = mybir.ActivationFunctionType.Exp
    Relu = mybir.ActivationFunctionType.Relu
    Prelu = mybir.ActivationFunctionType.Prelu
    Alu = mybir.AluOpType

    nc = bacc.Bacc("TRN2", target_bir_lowering=False, debug=False)

    # adjt8 row block jt: F1 tiles encoded {masked: -240, open: 0};
    # F3 tiles encoded {masked: 0, open: 1}
    adjt8 = nc.dram_tensor("adjt8", [N, R], fp8, kind="ExternalInput")
    ctxot = nc.dram_tensor("ctxot", [IN, R], bf16, kind="ExternalInput")
    rel_aug = nc.dram_tensor("rel_aug", [N, DA], bf16, kind="ExternalInput")
    vl_in = nc.dram_tensor("vl_in", [IN], bf16, kind="ExternalInput")
    vr_in = nc.dram_tensor("vr_in", [IN], bf16, kind="ExternalInput")
    # bias2 = [b_l + b_r, 0.2*(b_l + b_r)]
    bias2 = nc.dram_tensor("bias2", [2], f32, kind="ExternalInput")
    out = nc.dram_tensor("out", [R, D], f32, kind="ExternalOutput")
    l_scr = nc.dram_tensor("l_scr", [R], f32)
    a_scr = nc.dram_tensor("a_scr", [R], bf16)
    c_scr = nc.dram_tensor("c_scr", [R], bf16)
    r_own = nc.dram_tensor("r_own", [R], f32)
    r_gath = nc.dram_tensor("r_gath", [N], f32)

    t_sched = t_sched_for(cfg)  # True -> F1 (ACT Prelu+Exp, CCE-add mask)

    with tile.TileContext(nc) as tc, ExitStack() as ctx:
        singles = ctx.enter_context(tc.tile_pool(name="singles", bufs=1))
        adj_pool = ctx.enter_context(
            tc.tile_pool(name="adjp", bufs=cfg.get("adj_bufs", 8)))
        lk_pool = ctx.enter_context(
            tc.tile_pool(name="lkp", bufs=cfg.get("lk_bufs", 6)))
        t_pool = ctx.enter_context(
            tc.tile_pool(name="tp", bufs=cfg.get("t_bufs", 6)))
        zm_pool = ctx.enter_context(
            tc.tile_pool(name="zmp", bufs=cfg.get("zm_bufs", 8)))
        out_pool = ctx.enter_context(tc.tile_pool(name="outp", bufs=4))
        sm_pool = ctx.enter_context(tc.tile_pool(name="smp", bufs=4))
        acc_psum = ctx.enter_context(
            tc.tile_pool(name="accps", bufs=1, space="PSUM"))

        def acc_tile(b):
            return acc_psum.tile([P, 512], f32, tag=f"bank{b}", name=f"bank{b}")

        prolog = {}

        def _emit_prologue():
            # Runs ONCE (outside the reps loop): collectives cannot sit in
            # a hardware loop (NRT straight-line collective ordering).
            vrT = singles.tile([P, KT], bf16)
            nc.sync.dma_start(
                out=vrT, in_=bass.AP(tensor=vr_in, offset=0, ap=[[1, P], [P, KT]])
            )
            own_ctx = singles.tile([P, KT, R], bf16)
            nc.sync.dma_start(
                out=own_ctx,
                in_=bass.AP(tensor=ctxot, offset=0,
                            ap=[[R, P], [P * R, KT], [1, R]]),
            )
            ones_col = singles.tile([P, 1], bf16)
            nc.vector.memset(ones_col[:], 1.0)
            # warm the ACT table set early (implicit LoadActFuncSet)
            warm = singles.tile([P, 1], bf16)
            nc.scalar.activation(warm, ones_col, Exp, bias=0.0, scale=1.0)

            # right dots for own columns -> r_own -> AllGather -> r_gath
            Copy = mybir.ActivationFunctionType.Copy
            dotA = acc_tile(6)
            dotB = acc_tile(7)
            for half, dst in ((0, dotA), (1, dotB)):
                for kt in range(KT):
                    nc.tensor.matmul(
                        dst[0:1, 0:512],
                        lhsT=vrT[:, kt: kt + 1],
                        rhs=own_ctx[:, kt, half * 512: (half + 1) * 512],
                        start=(kt == 0),
                        stop=(kt == KT - 1),
                    )
                rrow = sm_pool.tile([1, 512], f32, tag="dotrow", name="rrow")
                nc.scalar.activation(rrow, dst[0:1, 0:512], Copy,
                                     bias=0.0, scale=1.0)
                nc.sync.dma_start(
                    out=bass.AP(tensor=r_own, offset=half * 512, ap=[[1, 512]]),
                    in_=rrow,
                )
            nc.gpsimd.collective_compute(
                "AllGather",
                Alu.bypass,
                replica_groups=[list(range(NCORES))],
                ins=[r_own.ap().opt()],
                outs=[r_gath.ap().opt()],
            )
            prolog["own_ctx"] = own_ctx

        def _emit_body():
            own_ctx = prolog["own_ctx"]
            # ---------------- singles / params ----------------
            vlT = singles.tile([P, KT], bf16)
            nc.sync.dma_start(
                out=vlT, in_=bass.AP(tensor=vl_in, offset=0, ap=[[1, P], [P, KT]])
            )
            b2 = singles.tile([P, 2], f32)
            nc.sync.dma_start(
                out=b2, in_=bass.AP(tensor=bias2, offset=0, ap=[[0, P], [1, 2]])
            )
            zeros_sb = singles.tile([P, DA], bf16)
            nc.vector.memset(zeros_sb[:], 0.0)
            zid = singles.tile([P, P], bf16)
            nc.vector.memset(zid[:], 0.0)

            # relation tiles [P, NJT, DA], loaded in nrel chunks
            rel_sb = singles.tile([P, NJT, DA], bf16)

            def emit_rel_chunk(rq):
                nc.sync.dma_start(
                    out=rel_sb[:, rq * relw: (rq + 1) * relw, :],
                    in_=bass.AP(tensor=rel_aug, offset=rq * relw * P * DA,
                                ap=[[DA, P], [P * DA, relw], [1, DA]]),
                )

            emit_rel_chunk(0)
            emit_rel_chunk(1)

            # ---------------- left dots (borrow acc banks) ----------
            Copy = mybir.ActivationFunctionType.Copy
            dotC = acc_tile(6)
            dotD = acc_tile(7)
            for half, dst in ((0, dotC), (1, dotD)):
                for kt in range(KT):
                    nc.tensor.matmul(
                        dst[0:1, 0:512],
                        lhsT=vlT[:, kt: kt + 1],
                        rhs=own_ctx[:, kt, half * 512: (half + 1) * 512],
                        start=(kt == 0),
                        stop=(kt == KT - 1),
                    )
                lrow = sm_pool.tile([1, 512], f32, tag="dotrow", name="lrow")
                nc.scalar.activation(lrow, dst[0:1, 0:512], Copy,
                                     bias=0.0, scale=1.0)
                nc.sync.dma_start(
                    out=bass.AP(tensor=l_scr, offset=half * 512, ap=[[1, 512]]),
                    in_=lrow,
                )

            # ---------------- left factors: column layout + exps --------
            l_col = singles.tile([P, NI], f32)
            nc.sync.dma_start(
                out=l_col, in_=bass.AP(tensor=l_scr, offset=0, ap=[[1, P], [P, NI]])
            )
            left_col = singles.tile([P, NI], f32)
            nc.vector.tensor_scalar_add(left_col, l_col, b2[:, 0:1])
            A_col = singles.tile([P, NI], bf16)
            nc.scalar.activation(A_col, left_col, Exp, bias=0.0, scale=1.0)
            C_col = singles.tile([P, NI], bf16)
            nc.scalar.activation(C_col, left_col, Exp, bias=0.0, scale=0.2)
            # bounce all three to DRAM, broadcast back along partitions
            nc.sync.dma_start(
                out=bass.AP(tensor=l_scr, offset=0, ap=[[1, P], [P, NI]]),
                in_=left_col[:, 0:NI],
            )
            nc.sync.dma_start(
                out=bass.AP(tensor=a_scr, offset=0, ap=[[1, P], [P, NI]]),
                in_=A_col[:, 0:NI],
            )
            nc.sync.dma_start(
                out=bass.AP(tensor=c_scr, offset=0, ap=[[1, P], [P, NI]]),
                in_=C_col[:, 0:NI],
            )
            left_bcast = singles.tile([P, R], f32)
            nc.sync.dma_start(
                out=left_bcast,
                in_=bass.AP(tensor=l_scr, offset=0, ap=[[0, P], [1, R]]),
            )
            A_bcast = singles.tile([P, R], bf16)
            nc.sync.dma_start(
                out=A_bcast,
                in_=bass.AP(tensor=a_scr, offset=0, ap=[[0, P], [1, R]]),
            )
            C_bcast = singles.tile([P, R], bf16)
            nc.sync.dma_start(
                out=C_bcast,
                in_=bass.AP(tensor=c_scr, offset=0, ap=[[0, P], [1, R]]),
            )

            # ---------------- right factors (post-gather) ---------------
            r_all = singles.tile([P, NJT], f32)
            # gpsimd queue: FIFO-ordered after the prologue's collective
            nc.gpsimd.dma_start(
                out=r_all, in_=bass.AP(tensor=r_gath, offset=0,
                                       ap=[[1, P], [P, NJT]])
            )
            B_all = singles.tile([P, NJT], f32)
            nc.scalar.activation(B_all, r_all, Exp, bias=0.0, scale=1.0)
            D_all = singles.tile([P, NJT], f32)
            nc.scalar.activation(D_all, r_all, Exp, bias=0.0, scale=0.2)

            # ---------------- acc banks: zero, then accumulate ----------
            accs = []
            for b in range(8):
                t_ = acc_tile(b)  # banks 6,7: version 3 (after dots)
                nc.tensor.matmul(
                    t_[:, 0:DA], lhsT=zid[:], rhs=zeros_sb[:, 0:DA],
                    start=True, stop=True,
                )
                accs.append(t_)

            # ---------------- main loop helpers ----------------
            def emit_mms(zm, jt):
                for ib in range(NI):
                    nc.tensor.matmul(
                        accs[ib][:, 0:DA],
                        lhsT=zm[:, ib * P: (ib + 1) * P],
                        rhs=rel_sb[:, jt, :],
                        start=False,
                        stop=(jt == NJT - 1),
                    )

            def emit_f3_tile(jt, adj_ap):
                ts0 = t_pool.tile([P, R], bf16, tag="t0", name="t0")
                nc.vector.tensor_scalar(
                    ts0, A_bcast, B_all[:, jt: jt + 1], None, Alu.mult)
                ts1 = t_pool.tile([P, R], bf16, tag="t1", name="t1")
                nc.vector.tensor_scalar(
                    ts1, C_bcast, D_all[:, jt: jt + 1], None, Alu.mult)
                mt = t_pool.tile([P, R], bf16, tag="m", name="mt")
                nc.vector.tensor_max(mt, ts0, ts1)
                zm = zm_pool.tile([P, R], bf16, tag="zm", name="zm")
                nc.vector.tensor_tensor(zm, mt, adj_ap, op=Alu.mult)
                emit_mms(zm, jt)

            if cfg.get("group4", False):
                # -------- grouped mode: 1 SWDGE trigger per 4 j-tiles ----
                ngrp = NJT // 4
                adjg_ahead = cfg.get("adjg_ahead", 2)
                f3_groups = [g for g in range(ngrp) if not t_sched[4 * g]]
                at_tiles = {}

                def emit_adj_grp(g):
                    at = adj_pool.tile([P, 4, R], bf16, tag="adj", name="at")
                    nc.gpsimd.dma_start(
                        out=at,
                        in_=bass.AP(tensor=adjt8, offset=4 * g * P * R,
                                    ap=[[R, P], [P * R, 4], [1, R]]),
                    )
                    at_tiles[g] = at

                for g in f3_groups[:adjg_ahead]:
                    emit_adj_grp(g)
                emitted = min(len(f3_groups), adjg_ahead)

                for g in range(ngrp):
                    jt0 = 4 * g
                    if jt0 % relw == 0 and jt0 // relw + 2 < nrel:
                        emit_rel_chunk(jt0 // relw + 2)
                    if t_sched[jt0]:
                        lk4 = lk_pool.tile([P, 4, R], bf16, tag="lk", name="lk")
                        for k in range(4):
                            nc.scalar.activation(
                                lk4[:, k, :], left_bcast, Prelu,
                                bias=r_all[:, jt0 + k: jt0 + k + 1],
                                scale=1.0, alpha=0.2,
                            )
                        nc.gpsimd.dma_start(
                            out=lk4,
                            in_=bass.AP(tensor=adjt8, offset=jt0 * P * R,
                                        ap=[[R, P], [P * R, 4], [1, R]]),
                            accum_op=Alu.add,
                        )
                        for k in range(4):
                            zm = zm_pool.tile([P, R], bf16, tag="zm", name="zm")
                            nc.scalar.activation(
                                zm, lk4[:, k, :], Exp, bias=0.0, scale=1.0)
                            emit_mms(zm, jt0 + k)
                    else:
                        if emitted < len(f3_groups):
                            emit_adj_grp(f3_groups[emitted])
                            emitted += 1
                        at = at_tiles.pop(g)
                        for k in range(4):
                            emit_f3_tile(jt0 + k, at[:, k, :])
            else:
                # -------- per-tile mode ----------------------------------
                adj_tiles = {}

                def emit_adj_f3(jt):
                    at = adj_pool.tile([P, R], bf16, tag="adj", name="at")
                    nc.gpsimd.dma_start(
                        out=at,
                        in_=bass.AP(tensor=adjt8, offset=jt * P * R,
                                    ap=[[R, P], [1, R]]),
                    )
                    adj_tiles[jt] = at

                f3_list = [j for j in range(NJT) if not t_sched[j]]
                for jt in f3_list[:adj_ahead]:
                    emit_adj_f3(jt)
                f3_emitted = min(len(f3_list), adj_ahead)

                for jt in range(NJT):
                    if jt % relw == relw - 2 and 2 + jt // relw < nrel:
                        emit_rel_chunk(2 + jt // relw)

                    if t_sched[jt]:
                        # F1: Prelu -> CCE-add(log-mask) -> Exp
                        lk = lk_pool.tile([P, R], bf16, tag="lk", name="lk")
                        nc.scalar.activation(
                            lk, left_bcast, Prelu,
                            bias=r_all[:, jt: jt + 1], scale=1.0, alpha=0.2,
                        )
                        nc.gpsimd.dma_start(
                            out=lk,
                            in_=bass.AP(tensor=adjt8, offset=jt * P * R,
                                        ap=[[R, P], [1, R]]),
                            accum_op=Alu.add,
                        )
                        zm = zm_pool.tile([P, R], bf16, tag="zm", name="zm")
                        nc.scalar.activation(zm, lk, Exp, bias=0.0, scale=1.0)
                        emit_mms(zm, jt)
                    else:
                        if f3_emitted < len(f3_list):
                            emit_adj_f3(f3_list[f3_emitted])
                            f3_emitted += 1
                        emit_f3_tile(jt, adj_tiles.pop(jt))

            # ---------------- finalize ----------------
            for ib in range(NI):
                recip = sm_pool.tile([P, 1], f32, tag="recip", name="recip")
                nc.vector.reciprocal(recip, accs[ib][:, D: D + 1])
                ob = out_pool.tile([P, D], f32, tag="ob", name="ob")
                nc.scalar.activation(
                    ob, accs[ib][:, 0:D], Relu, bias=0.0, scale=recip[:, 0:1]
                )
                nc.sync.dma_start(out=out[ib * P: (ib + 1) * P, :], in_=ob)

        _emit_prologue()
        if reps > 1:
            with tc.For_i(0, reps, 1):
                _emit_body()
        else:
            _emit_body()

    nc.compile()
    return nc


_BASE_CFG = dict(g_act=0.45)


def _get_program(cfg_key):
    if cfg_key not in _CACHE:
        _CACHE[cfg_key] = build_program(dict(_BASE_CFG))
    return _CACHE[cfg_key]


def prepare_in_maps(relation, context, adj_tensor, W_common, w_left, b_left,
                    w_right, b_right):
    bf = ml_dtypes.bfloat16
    e4 = ml_dtypes.float8_e4m3
    relation = np.asarray(relation, dtype=np.float32)
    context = np.asarray(context, dtype=np.float32)
    adj_tensor = np.asarray(adj_tensor, dtype=np.float32)
    W_common = np.asarray(W_common, dtype=np.float32)
    w_left = np.asarray(w_left, dtype=np.float32)
    w_right = np.asarray(w_right, dtype=np.float32)
    b_l = float(np.asarray(b_left))
    b_r = float(np.asarray(b_right))

    # host-side parameter folding (weights only, no activations)
    v_left = (W_common.T @ w_left).astype(bf)
    v_right = (W_common.T @ w_right).astype(bf)
    b2 = b_l + b_r
    bias2 = np.array([b2, 0.2 * b2], dtype=np.float32)

    rel_aug = np.ones((N, DA), dtype=np.float32)
    rel_aug[:, 0:D] = relation
    rel_aug = rel_aug.astype(bf)
    ctx_t = np.ascontiguousarray(context.T).astype(bf)  # [IN, N]

    t_sched = t_sched_for(_BASE_CFG)
    # per-j-tile adjacency encoding (row blocks of adjt8 = j-tiles)
    open_v = np.empty(NJT, dtype=np.float32)
    masked_v = np.empty(NJT, dtype=np.float32)
    for jt in range(NJT):
        if t_sched[jt]:  # F1: log-mask add
            open_v[jt] = 0.0
            masked_v[jt] = -240.0
        else:  # F3: multiplicative mask
            open_v[jt] = 1.0
            masked_v[jt] = 0.0
    open_col = np.repeat(open_v, P)[:, None]     # [N, 1]
    masked_col = np.repeat(masked_v, P)[:, None]

    in_maps = []
    for c in range(NCORES):
        sl = slice(c * R, (c + 1) * R)
        adjt = np.ascontiguousarray(adj_tensor[sl].T)  # [N, R]
        enc = np.where(adjt > 0, open_col, masked_col).astype(e4)
        in_maps.append({
            "adjt8": enc,
            "ctxot": np.ascontiguousarray(ctx_t[:, sl]),
            "rel_aug": rel_aug,
            "vl_in": v_left,
            "vr_in": v_right,
            "bias2": bias2,
        })
    return in_maps


# ------------------------------------------------------------------- entry
def kernel(relation, context, adj_tensor, W_common, w_left, b_left, w_right,
           b_right):
    from concourse.bass_utils import run_bass_kernel_spmd

    in_maps = prepare_in_maps(relation, context, adj_tensor, W_common,
                              w_left, b_left, w_right, b_right)
    nc = _get_program("main")
    last_err = None
    for _attempt in range(3):
        try:
            res = run_bass_kernel_spmd(nc, in_maps, list(range(NCORES)))
            outs = [res.results[c]["out"] for c in range(NCORES)]
            return np.concatenate(outs, axis=0).astype(np.float32)
        except Exception as e:  # transient device-unrecoverable seen on axon
            last_err = e
            import time as _time

            try:
                import jax

                jax.clear_caches()
            except Exception:
                pass
            _time.sleep(3.0)
    raise last_err


# revision 29
# speedup vs baseline: 2.9873x; 2.2684x over previous
"""Trainium2 Bass kernel for nn_DenseAttentionLayer (gnn_message_passing) — v4.

Math (reference):
    in_fts = context @ W_common.T            # (N, HID)
    left   = in_fts @ w_left + b_left        # (N,)
    right  = in_fts @ w_right + b_right      # (N,)
    logits = leaky_relu(left[:,None] + right[None,:], 0.2)
    logits = where(adj <= 0, -inf, logits)
    coefs  = softmax(logits, axis=-1)
    out    = relu(coefs @ relation)          # (N, REL_DIM)

v4 design (changes vs v3):
  * Mask in LOG space for ACT-form tiles: the host encodes the adjacency
    as fp8 {0, -240}; a software-DGE CCE-ADD DMA adds it into the Prelu
    output before Exp, so exp() underflows to 0 on masked entries. The
    mask costs zero DVE/ACT work on those tiles and the adjacency
    crosses HBM as 1 byte/elem.
  * DVE-form tiles read the adjacency as {0,1} fp8 via a cast DMA
    (fp8 in HBM -> bf16 in SBUF) and apply one tensor_tensor mult.
  * Denominators ride the main matmul: the relation is augmented with a
    ones column (rhs [128, 257]); acc column 256 accumulates the
    softmax denominator. Removes 512 single-column matmuls + their
    weight loads per core.
  * left/right dot products via wide rhs matmuls (lhsT = v [128,1],
    rhs = ctx chunk [128, 512]) — 16x fewer PE instructions than
    column-form dots.
  * Right factors are computed only for the core's own 1024 columns and
    AllGathered (DRAM collective) — the replicated 8.4MB ctxt read is
    gone entirely.
  * PSUM: 8 banks hold the 8 i-block accumulators [128, 257]; the
    startup dots borrow two banks (tag-versioned) before they are
    zeroed for accumulation.

Sharding: row-shard the N x N logits; core c owns output rows
[c*R, (c+1)*R), R = N/8.  All params replicated; context sharded for
the right factors (own slice only).
"""

import os
import sys

for _p in ("/opt/trn_rl_repo",):
    if _p not in sys.path and os.path.isdir(_p):
        sys.path.insert(0, _p)

from contextlib import ExitStack

import ml_dtypes
import numpy as np

# ---------------------------------------------------------------- constants
N = 8192  # num relations (columns j)
IN = 512  # context feature dim
D = 256  # relation dim (output dim)
DA = D + 1  # + ones column (softmax denominator)
NCORES = 8
P = 128
R = N // NCORES  # rows per core (i range)
KT = IN // P  # IN tiles (4)
NJT = N // P  # 64 j-tiles
NI = R // P  # 8 i-blocks

_CACHE = {}


def _spread(frac, n):
    """n booleans with ~frac True, evenly spread."""
    out = []
    acc = 0.0
    for _ in range(n):
        acc += frac
        if acc >= 1.0 - 1e-9:
            acc -= 1.0
            out.append(True)
        else:
            out.append(False)
    return out


def t_sched_for(cfg):
    if cfg.get("group4", False):
        # group-aligned schedule: groups of 4 j-tiles share one form so a
        # single SWDGE trigger covers the whole group
        g = _spread(cfg.get("g_act", 0.45), NJT // 4)
        return [v for v in g for _ in range(4)]
    sched = _spread(cfg.get("g_act", 0.45), NJT)
    head = cfg.get("f1_head", 0)
    if head:
        # force the first `head` tiles to F1 (they only need left_bcast +
        # r_all, not the A/C broadcast bounce), keeping the F1 count fixed
        want = sum(sched)
        sched = [True] * head + sched[head:]
        i = NJT - 1
        while sum(sched) > want and i >= head:
            if sched[i]:
                sched[i] = False
            i -= 1
    return sched


# ------------------------------------------------------------------ builder
def build_program(cfg):
    import concourse.bass as bass
    import concourse.tile as tile
    from concourse import bacc, mybir

    f32 = mybir.dt.float32
    bf16 = mybir.dt.bfloat16
    fp8 = mybir.dt.float8e4

    reps = cfg.get("reps", 1)
    adj_ahead = cfg.get("adj_ahead", 6)  # j-tiles of adj prefetch for F3
    nrel = cfg.get("nrel", 8)  # rel load chunks
    relw = NJT // nrel

    Exp = mybir.ActivationFunctionType.Exp
    Relu = mybir.ActivationFunctionType.Relu
    Prelu = mybir.ActivationFunctionType.Prelu
    Alu = mybir.AluOpType

    nc = bacc.Bacc("TRN2", target_bir_lowering=False, debug=False)

    # adjt8 row block jt: F1 tiles encoded {masked: -240, open: 0};
    # F3 tiles encoded {masked: 0, open: 1}
    adjt8 = nc.dram_tensor("adjt8", [N, R], fp8, kind="ExternalInput")
    if cfg.get("f3_hwdge", False):
        adjt16 = nc.dram_tensor("adjt16", [N, R], bf16, kind="ExternalInput")
    ctxot = nc.dram_tensor("ctxot", [IN, R], bf16, kind="ExternalInput")
    rel_aug = nc.dram_tensor("rel_aug", [N, DA], bf16, kind="ExternalInput")
    vl_in = nc.dram_tensor("vl_in", [IN], bf16, kind="ExternalInput")
    vr_in = nc.dram_tensor("vr_in", [IN], bf16, kind="ExternalInput")
    # bias2 = [b_l + b_r, 0.2*(b_l + b_r)]
    bias2 = nc.dram_tensor("bias2", [2], f32, kind="ExternalInput")
    out = nc.dram_tensor("out", [R, D], f32, kind="ExternalOutput")
    l_scr = nc.dram_tensor("l_scr", [R], f32)
    a_scr = nc.dram_tensor("a_scr", [R], bf16)
    c_scr = nc.dram_tensor("c_scr", [R], bf16)
    r_own = nc.dram_tensor("r_own", [R], f32)
    r_gath = nc.dram_tensor("r_gath", [N], f32)

    t_sched = t_sched_for(cfg)  # True -> F1 (ACT Prelu+Exp, CCE-add mask)

    with tile.TileContext(nc) as tc, ExitStack() as ctx:
        singles = ctx.enter_context(tc.tile_pool(name="singles", bufs=1))
        adj_pool = ctx.enter_context(
            tc.tile_pool(name="adjp", bufs=cfg.get("adj_bufs", 8)))
        lk_pool = ctx.enter_context(
            tc.tile_pool(name="lkp", bufs=cfg.get("lk_bufs", 6)))
        t_pool = ctx.enter_context(
            tc.tile_pool(name="tp", bufs=cfg.get("t_bufs", 6)))
        zm_pool = ctx.enter_context(
            tc.tile_pool(name="zmp", bufs=cfg.get("zm_bufs", 8)))
        out_pool = ctx.enter_context(tc.tile_pool(name="outp", bufs=4))
        sm_pool = ctx.enter_context(tc.tile_pool(name="smp", bufs=4))
        acc_psum = ctx.enter_context(
            tc.tile_pool(name="accps", bufs=1, space="PSUM"))

        def acc_tile(b):
            return acc_psum.tile([P, 512], f32, tag=f"bank{b}", name=f"bank{b}")

        prolog = {}

        def _emit_prologue():
            # Runs ONCE (outside the reps loop): collectives cannot sit in
            # a hardware loop (NRT straight-line collective ordering).
            vrT = singles.tile([P, KT], bf16)
            nc.sync.dma_start(
                out=vrT, in_=bass.AP(tensor=vr_in, offset=0, ap=[[1, P], [P, KT]])
            )
            own_ctx = singles.tile([P, KT, R], bf16)
            nc.sync.dma_start(
                out=own_ctx,
                in_=bass.AP(tensor=ctxot, offset=0,
                            ap=[[R, P], [P * R, KT], [1, R]]),
            )
            ones_col = singles.tile([P, 1], bf16)
            nc.vector.memset(ones_col[:], 1.0)
            # warm the ACT table set early (implicit LoadActFuncSet)
            warm = singles.tile([P, 1], bf16)
            nc.scalar.activation(warm, ones_col, Exp, bias=0.0, scale=1.0)

            # right dots for own columns -> r_own -> AllGather -> r_gath
            Copy = mybir.ActivationFunctionType.Copy
            dotA = acc_tile(6)
            dotB = acc_tile(7)
            for half, dst in ((0, dotA), (1, dotB)):
                for kt in range(KT):
                    nc.tensor.matmul(
                        dst[0:1, 0:512],
                        lhsT=vrT[:, kt: kt + 1],
                        rhs=own_ctx[:, kt, half * 512: (half + 1) * 512],
                        start=(kt == 0),
                        stop=(kt == KT - 1),
                    )
                rrow = sm_pool.tile([1, 512], f32, tag="dotrow", name="rrow")
                nc.scalar.activation(rrow, dst[0:1, 0:512], Copy,
                                     bias=0.0, scale=1.0)
                nc.sync.dma_start(
                    out=bass.AP(tensor=r_own, offset=half * 512, ap=[[1, 512]]),
                    in_=rrow,
                )
            nc.gpsimd.collective_compute(
                "AllGather",
                Alu.bypass,
                replica_groups=[list(range(NCORES))],
                ins=[r_own.ap().opt()],
                outs=[r_gath.ap().opt()],
            )
            prolog["own_ctx"] = own_ctx

        def _emit_body():
            own_ctx = prolog["own_ctx"]
            # ---------------- singles / params ----------------
            vlT = singles.tile([P, KT], bf16)
            nc.sync.dma_start(
                out=vlT, in_=bass.AP(tensor=vl_in, offset=0, ap=[[1, P], [P, KT]])
            )
            b2 = singles.tile([P, 2], f32)
            nc.sync.dma_start(
                out=b2, in_=bass.AP(tensor=bias2, offset=0, ap=[[0, P], [1, 2]])
            )
            zeros_sb = singles.tile([P, DA], bf16)
            nc.vector.memset(zeros_sb[:], 0.0)
            zeros_f = singles.tile([P, D], f32)
            nc.vector.memset(zeros_f[:], 0.0)
            zid = singles.tile([P, P], bf16)
            nc.vector.memset(zid[:], 0.0)

            # relation tiles [P, NJT, DA], loaded in nrel chunks
            rel_sb = singles.tile([P, NJT, DA], bf16)

            def emit_rel_chunk(rq):
                nc.sync.dma_start(
                    out=rel_sb[:, rq * relw: (rq + 1) * relw, :],
                    in_=bass.AP(tensor=rel_aug, offset=rq * relw * P * DA,
                                ap=[[DA, P], [P * DA, relw], [1, DA]]),
                )

            emit_rel_chunk(0)
            emit_rel_chunk(1)

            # ---------------- left dots (borrow acc banks) ----------
            Copy = mybir.ActivationFunctionType.Copy
            dotC = acc_tile(6)
            dotD = acc_tile(7)
            for half, dst in ((0, dotC), (1, dotD)):
                for kt in range(KT):
                    nc.tensor.matmul(
                        dst[0:1, 0:512],
                        lhsT=vlT[:, kt: kt + 1],
                        rhs=own_ctx[:, kt, half * 512: (half + 1) * 512],
                        start=(kt == 0),
                        stop=(kt == KT - 1),
                    )
                if not cfg.get("lbcast_pe", False):
                    lrow = sm_pool.tile([1, 512], f32, tag="dotrow", name="lrow")
                    nc.scalar.activation(lrow, dst[0:1, 0:512], Copy,
                                         bias=0.0, scale=1.0)
                    nc.sync.dma_start(
                        out=bass.AP(tensor=l_scr, offset=half * 512,
                                    ap=[[1, 512]]),
                        in_=lrow,
                    )

            # ---------------- left factors -> broadcast tiles ------------
            if cfg.get("lbcast_pe", False):
                # PE broadcast: left row -> [128, R] via ones-lhsT matmul;
                # avoids the 3x DRAM bounce round-trips
                ones_row = singles.tile([1, P], bf16)
                nc.vector.memset(ones_row[:], 1.0)
                lrow = singles.tile([1, R], bf16)
                for half, dst in ((0, dotC), (1, dotD)):
                    nc.vector.tensor_scalar_add(
                        lrow[0:1, half * 512: (half + 1) * 512],
                        dst[0:1, 0:512], b2[0:1, 0:1],
                    )
                blA = acc_tile(4)
                blB = acc_tile(5)
                for half, bl in ((0, blA), (1, blB)):
                    nc.tensor.matmul(
                        bl[:, 0:512],
                        lhsT=ones_row[0:1, :],
                        rhs=lrow[0:1, half * 512: (half + 1) * 512],
                        start=True, stop=True,
                    )
                left_bcast = singles.tile([P, R], bf16)
                A_bcast = singles.tile([P, R], bf16)
                C_bcast = singles.tile([P, R], bf16)
                for half, bl in ((0, blA), (1, blB)):
                    sl = slice(half * 512, (half + 1) * 512)
                    nc.vector.tensor_copy(left_bcast[:, sl], bl[:, 0:512])
                    nc.scalar.activation(
                        A_bcast[:, sl], bl[:, 0:512], Exp, bias=0.0, scale=1.0)
                    nc.scalar.activation(
                        C_bcast[:, sl], bl[:, 0:512], Exp, bias=0.0, scale=0.2)
            else:
                l_col = singles.tile([P, NI], f32)
                nc.sync.dma_start(
                    out=l_col,
                    in_=bass.AP(tensor=l_scr, offset=0, ap=[[1, P], [P, NI]])
                )
                left_col = singles.tile([P, NI], f32)
                nc.vector.tensor_scalar_add(left_col, l_col, b2[:, 0:1])
                A_col = singles.tile([P, NI], bf16)
                nc.scalar.activation(A_col, left_col, Exp, bias=0.0, scale=1.0)
                C_col = singles.tile([P, NI], bf16)
                nc.scalar.activation(C_col, left_col, Exp, bias=0.0, scale=0.2)
                # bounce all three to DRAM, broadcast back along partitions
                nc.sync.dma_start(
                    out=bass.AP(tensor=l_scr, offset=0, ap=[[1, P], [P, NI]]),
                    in_=left_col[:, 0:NI],
                )
                nc.sync.dma_start(
                    out=bass.AP(tensor=a_scr, offset=0, ap=[[1, P], [P, NI]]),
                    in_=A_col[:, 0:NI],
                )
                nc.sync.dma_start(
                    out=bass.AP(tensor=c_scr, offset=0, ap=[[1, P], [P, NI]]),
                    in_=C_col[:, 0:NI],
                )
                left_bcast = singles.tile([P, R], f32)
                nc.sync.dma_start(
                    out=left_bcast,
                    in_=bass.AP(tensor=l_scr, offset=0, ap=[[0, P], [1, R]]),
                )
                A_bcast = singles.tile([P, R], bf16)
                nc.sync.dma_start(
                    out=A_bcast,
                    in_=bass.AP(tensor=a_scr, offset=0, ap=[[0, P], [1, R]]),
                )
                C_bcast = singles.tile([P, R], bf16)
                nc.sync.dma_start(
                    out=C_bcast,
                    in_=bass.AP(tensor=c_scr, offset=0, ap=[[0, P], [1, R]]),
                )

            # ---------------- right factors (post-gather) ---------------
            r_all = singles.tile([P, NJT], f32)
            # gpsimd queue: FIFO-ordered after the prologue's collective
            nc.gpsimd.dma_start(
                out=r_all, in_=bass.AP(tensor=r_gath, offset=0,
                                       ap=[[1, P], [P, NJT]])
            )
            B_all = singles.tile([P, NJT], f32)
            nc.scalar.activation(B_all, r_all, Exp, bias=0.0, scale=1.0)
            D_all = singles.tile([P, NJT], f32)
            nc.scalar.activation(D_all, r_all, Exp, bias=0.0, scale=0.2)

            # ---------------- acc banks: zero, then accumulate ----------
            accs = []
            for b in range(8):
                t_ = acc_tile(b)  # banks 6,7: version 3 (after dots)
                nc.tensor.matmul(
                    t_[:, 0:DA], lhsT=zid[:], rhs=zeros_sb[:, 0:DA],
                    start=True, stop=True,
                )
                accs.append(t_)

            # ---------------- main loop helpers ----------------
            def emit_mms(zm, jt):
                if cfg.get("ablate_mm", False):
                    if jt != NJT - 1:
                        return
                for ib in range(NI):
                    nc.tensor.matmul(
                        accs[ib][:, 0:DA],
                        lhsT=zm[:, ib * P: (ib + 1) * P],
                        rhs=rel_sb[:, jt, :],
                        start=False,
                        stop=(jt == NJT - 1),
                    )

            def emit_f3_tile(jt, adj_ap):
                if cfg.get("ablate_ew", False):
                    zm = zm_pool.tile([P, R], bf16, tag="zm", name="zm")
                    nc.vector.tensor_tensor(zm, A_bcast, adj_ap, op=Alu.mult)
                    emit_mms(zm, jt)
                    return
                if cfg.get("stt", True):
                    # fused: u = (C*D_j) max (A*B_j); zm = u * adj
                    ts0 = t_pool.tile([P, R], bf16, tag="t0", name="t0")
                    nc.vector.tensor_scalar(
                        ts0, A_bcast, B_all[:, jt: jt + 1], None, Alu.mult)
                    mt = t_pool.tile([P, R], bf16, tag="m", name="mt")
                    nc.vector.scalar_tensor_tensor(
                        mt, C_bcast, D_all[:, jt: jt + 1], ts0,
                        op0=Alu.mult, op1=Alu.max)
                else:
                    ts0 = t_pool.tile([P, R], bf16, tag="t0", name="t0")
                    nc.vector.tensor_scalar(
                        ts0, A_bcast, B_all[:, jt: jt + 1], None, Alu.mult)
                    ts1 = t_pool.tile([P, R], bf16, tag="t1", name="t1")
                    nc.vector.tensor_scalar(
                        ts1, C_bcast, D_all[:, jt: jt + 1], None, Alu.mult)
                    mt = t_pool.tile([P, R], bf16, tag="m", name="mt")
                    nc.vector.tensor_max(mt, ts0, ts1)
                zm = zm_pool.tile([P, R], bf16, tag="zm", name="zm")
                nc.vector.tensor_tensor(zm, mt, adj_ap, op=Alu.mult)
                emit_mms(zm, jt)

            if cfg.get("group4", False):
                # -------- grouped mode: 1 SWDGE trigger per 4 j-tiles ----
                ngrp = NJT // 4
                adjg_ahead = cfg.get("adjg_ahead", 2)
                f3_groups = [g for g in range(ngrp) if not t_sched[4 * g]]
                at_tiles = {}

                def emit_adj_grp(g):
                    at = adj_pool.tile([P, 4, R], bf16, tag="adj", name="at")
                    if cfg.get("f3_hwdge", False):
                        nc.sync.dma_start(
                            out=at,
                            in_=bass.AP(tensor=adjt16, offset=4 * g * P * R,
                                        ap=[[R, P], [P * R, 4], [1, R]]),
                        )
                    else:
                        nc.gpsimd.dma_start(
                            out=at,
                            in_=bass.AP(tensor=adjt8, offset=4 * g * P * R,
                                        ap=[[R, P], [P * R, 4], [1, R]]),
                        )
                    at_tiles[g] = at

                for g in f3_groups[:adjg_ahead]:
                    emit_adj_grp(g)
                emitted = min(len(f3_groups), adjg_ahead)

                for g in range(ngrp):
                    jt0 = 4 * g
                    if jt0 % relw == 0 and jt0 // relw + 2 < nrel:
                        emit_rel_chunk(jt0 // relw + 2)
                    if t_sched[jt0]:
                        lk4 = lk_pool.tile([P, 4, R], bf16, tag="lk", name="lk")
                        for k in range(4):
                            nc.scalar.activation(
                                lk4[:, k, :], left_bcast, Prelu,
                                bias=r_all[:, jt0 + k: jt0 + k + 1],
                                scale=1.0, alpha=0.2,
                            )
                        nc.gpsimd.dma_start(
                            out=lk4,
                            in_=bass.AP(tensor=adjt8, offset=jt0 * P * R,
                                        ap=[[R, P], [P * R, 4], [1, R]]),
                            accum_op=Alu.add,
                        )
                        for k in range(4):
                            zm = zm_pool.tile([P, R], bf16, tag="zm", name="zm")
                            nc.scalar.activation(
                                zm, lk4[:, k, :], Exp, bias=0.0, scale=1.0)
                            emit_mms(zm, jt0 + k)
                    else:
                        if emitted < len(f3_groups):
                            emit_adj_grp(f3_groups[emitted])
                            emitted += 1
                        at = at_tiles.pop(g)
                        for k in range(4):
                            emit_f3_tile(jt0 + k, at[:, k, :])
            else:
                # -------- per-tile mode ----------------------------------
                adj_tiles = {}

                def emit_adj_f3(jt):
                    at = adj_pool.tile([P, R], bf16, tag="adj", name="at")
                    if cfg.get("f3_hwdge", False):
                        nc.sync.dma_start(
                            out=at,
                            in_=bass.AP(tensor=adjt16, offset=jt * P * R,
                                        ap=[[R, P], [1, R]]),
                        )
                    else:
                        nc.gpsimd.dma_start(
                            out=at,
                            in_=bass.AP(tensor=adjt8, offset=jt * P * R,
                                        ap=[[R, P], [1, R]]),
                        )
                    adj_tiles[jt] = at

                if cfg.get("ablate_f3dma", False):
                    def emit_adj_f3(jt):  # noqa: F811
                        adj_tiles[jt] = A_bcast

                f3_list = [j for j in range(NJT) if not t_sched[j]]
                for jt in f3_list[:adj_ahead]:
                    emit_adj_f3(jt)
                f3_emitted = min(len(f3_list), adj_ahead)

                for jt in range(NJT):
                    if jt % relw == relw - 2 and 2 + jt // relw < nrel:
                        emit_rel_chunk(2 + jt // relw)

                    if t_sched[jt]:
                        # F1: Prelu -> CCE-add(log-mask) -> Exp
                        lk = lk_pool.tile([P, R], bf16, tag="lk", name="lk")
                        nc.scalar.activation(
                            lk, left_bcast, Prelu,
                            bias=r_all[:, jt: jt + 1], scale=1.0, alpha=0.2,
                        )
                        if not cfg.get("ablate_f1add", False):
                            nc.gpsimd.dma_start(
                                out=lk,
                                in_=bass.AP(tensor=adjt8, offset=jt * P * R,
                                            ap=[[R, P], [1, R]]),
                                accum_op=Alu.add,
                            )
                        zm = zm_pool.tile([P, R], bf16, tag="zm", name="zm")
                        nc.scalar.activation(zm, lk, Exp, bias=0.0, scale=1.0)
                        emit_mms(zm, jt)
                    else:
                        if f3_emitted < len(f3_list):
                            emit_adj_f3(f3_list[f3_emitted])
                            f3_emitted += 1
                        emit_f3_tile(jt, adj_tiles.pop(jt))

            # ---------------- finalize ----------------
            for ib in range(NI):
                recip = sm_pool.tile([P, 1], f32, tag="recip", name="recip")
                nc.vector.reciprocal(recip, accs[ib][:, D: D + 1])
                ob = out_pool.tile([P, D], f32, tag="ob", name="ob")
                if cfg.get("fin_dve", False):
                    # relu(acc*recip) = (acc*recip) max 0 in one DVE op
                    nc.vector.scalar_tensor_tensor(
                        ob, accs[ib][:, 0:D], recip[:, 0:1], zeros_f,
                        op0=Alu.mult, op1=Alu.max)
                else:
                    nc.scalar.activation(
                        ob, accs[ib][:, 0:D], Relu, bias=0.0,
                        scale=recip[:, 0:1]
                    )
                nc.sync.dma_start(out=out[ib * P: (ib + 1) * P, :], in_=ob)

        _emit_prologue()
        unroll = cfg.get("unroll", 1)
        if reps > 1:
            assert reps % unroll == 0
            with tc.For_i(0, reps // unroll, 1):
                for _ in range(unroll):
                    _emit_body()
        else:
            _emit_body()

    nc.compile()
    return nc


_BASE_CFG = dict(g_act=0.42, lbcast_pe=1, fin_dve=1)


def _get_program(cfg_key):
    if cfg_key not in _CACHE:
        _CACHE[cfg_key] = build_program(dict(_BASE_CFG))
    return _CACHE[cfg_key]


def prepare_in_maps(relation, context, adj_tensor, W_common, w_left, b_left,
                    w_right, b_right):
    bf = ml_dtypes.bfloat16
    e4 = ml_dtypes.float8_e4m3
    relation = np.asarray(relation, dtype=np.float32)
    context = np.asarray(context, dtype=np.float32)
    adj_tensor = np.asarray(adj_tensor, dtype=np.float32)
    W_common = np.asarray(W_common, dtype=np.float32)
    w_left = np.asarray(w_left, dtype=np.float32)
    w_right = np.asarray(w_right, dtype=np.float32)
    b_l = float(np.asarray(b_left))
    b_r = float(np.asarray(b_right))

    # host-side parameter folding (weights only, no activations)
    v_left = (W_common.T @ w_left).astype(bf)
    v_right = (W_common.T @ w_right).astype(bf)
    b2 = b_l + b_r
    bias2 = np.array([b2, 0.2 * b2], dtype=np.float32)

    rel_aug = np.ones((N, DA), dtype=np.float32)
    rel_aug[:, 0:D] = relation
    rel_aug = rel_aug.astype(bf)
    ctx_t = np.ascontiguousarray(context.T).astype(bf)  # [IN, N]

    f3_hwdge = _BASE_CFG.get("f3_hwdge", False)
    if f3_hwdge:
        # adjt8 = log encoding everywhere; adjt16 = 0/1 bf16 everywhere
        open_col = np.zeros((N, 1), dtype=np.float32)
        masked_col = np.full((N, 1), -240.0, dtype=np.float32)
    else:
        t_sched = t_sched_for(_BASE_CFG)
        # per-j-tile adjacency encoding (row blocks of adjt8 = j-tiles)
        open_v = np.empty(NJT, dtype=np.float32)
        masked_v = np.empty(NJT, dtype=np.float32)
        for jt in range(NJT):
            if t_sched[jt]:  # F1: log-mask add
                open_v[jt] = 0.0
                masked_v[jt] = -240.0
            else:  # F3: multiplicative mask
                open_v[jt] = 1.0
                masked_v[jt] = 0.0
        open_col = np.repeat(open_v, P)[:, None]     # [N, 1]
        masked_col = np.repeat(masked_v, P)[:, None]

    in_maps = []
    for c in range(NCORES):
        sl = slice(c * R, (c + 1) * R)
        adjt = np.ascontiguousarray(adj_tensor[sl].T)  # [N, R]
        m = {
            "adjt8": np.where(adjt > 0, open_col, masked_col).astype(e4),
            "ctxot": np.ascontiguousarray(ctx_t[:, sl]),
            "rel_aug": rel_aug,
            "vl_in": v_left,
            "vr_in": v_right,
            "bias2": bias2,
        }
        if f3_hwdge:
            m["adjt16"] = adjt.astype(bf)
        in_maps.append(m)
    return in_maps


# ------------------------------------------------------------------- entry
def kernel(relation, context, adj_tensor, W_common, w_left, b_left, w_right,
           b_right):
    from concourse.bass_utils import run_bass_kernel_spmd

    in_maps = prepare_in_maps(relation, context, adj_tensor, W_common,
                              w_left, b_left, w_right, b_right)
    nc = _get_program("main")
    last_err = None
    for _attempt in range(3):
        try:
            res = run_bass_kernel_spmd(nc, in_maps, list(range(NCORES)))
            outs = [res.results[c]["out"] for c in range(NCORES)]
            return np.concatenate(outs, axis=0).astype(np.float32)
        except Exception as e:  # transient device-unrecoverable seen on axon
            last_err = e
            import time as _time

            try:
                import jax

                jax.clear_caches()
            except Exception:
                pass
            _time.sleep(3.0)
    raise last_err
